# revision 26
# baseline (speedup 1.0000x reference)
"""Bidirectional LSTM (S=2048, B=4096, I=1, H=8, O=1) on 8 Trainium2 NeuronCores.

Strategy
--------
Pure data parallel over batch (512 rows/core) + sequence chunking with warmup:
a chunk started W=9 steps early from zero state converges to the exact
trajectory below fp16 noise.

Per core: NP=7 (fwd,bwd) chunk-stream pairs x G=3 pipelined rings (21 chunks
of l=98).  The scalar engine (ACT) is the bottleneck (~2.7us per ring-step);
3 rings keep it ~100% occupied, hiding the per-ring serial latency
(matmul -> sigmoid -> DVE cell math -> tanh(c) -> h' -> matmul, ~7us).
Streams stack block-diagonally in the matmul:
rhs = [h (112 rows) ; x (14) ; ones (1)] = [127, 512] fp16.  Per ring-step:

  PE : 4 fp16 matmuls into ONE merged [119,4,512] f32 PSUM tile
       (slices f,i,o on partitions 0:112; g + out-projection on 0:119).
       The g stationary is pre-scaled by 2 (tanh via sigmoid transport);
       its cols 112:119 compute sigmoid(2*(w_out.h + b_out)); biases ride
       the ones-row of the rhs.
  ACT: ONE sigmoid over the whole merged tile (FD=2048) + tanh(c) - the
       2-instruction-per-step floor for an LSTM on this engine.
  DVE: tg=2*sg-1 (tensor_scalar), tm=f*c, z=i*tg, c'=tm+z, h'=o*tanh(c')
       (fp16 2x perf mode).
  DMA: next x rows into the next rhs tile (sync queue); sigmoid-encoded out
       rows straight from the sigmoid-output SBUF tile to HBM (gpsimd
       queue), decoded host-side as 0.5*logit(y).

PSUM: 2 merged sets of 4 banks; 3 rings share them via round-parity
mapping set = (ring + round) % 2, which balances set pressure at 1.5
rings/set/round (a fixed ring->set map would serialize 2 full
quad+sigmoid holds on one set every round and stall).  Emission order
back(G-1,r-1), front(0), front(1), back(0), front(2), back(1) staggers the
rings across the in-order engine queues.

Warmup uses stationary copies with pair-0 gate columns zeroed so chunk 0
starts exactly from zero state.  A final flush round (R = l+W+1) emits the
last position's output, since out(pos p) rides the matmul of round p+W+1.

Measured: ~886 us HW exec across 8 cores (prior session 1.10 ms), rel err
5.7e-3 (fp16 datapath; fp32 reference tolerance 2e-2).
"""

import os
import sys

if "axon" not in os.environ.get("JAX_PLATFORMS", "axon"):
    os.environ["JAX_PLATFORMS"] = "axon,cpu"

try:
    import concourse  # noqa: F401
except ImportError:  # pragma: no cover
    sys.path.insert(0, "/opt/trn_rl_repo")

from contextlib import ExitStack

import numpy as np

import concourse.bacc as bacc
import concourse.mybir as mybir
import concourse.tile as tile

S, B, I, H, O = 2048, 4096, 1, 8, 1
N_CORES = 8
BC = B // N_CORES  # batch columns per core

NP = 7      # stream pairs per group
G = 3       # pipelined rings per core (share 2 PSUM sets: ring g -> set g%2)
W = 8       # warmup rounds per chunk

KH = 16 * NP           # h rows / gate partitions (112)
KR = KH + 2 * NP + 1   # rhs rows: h + x + ones (127)
KO = KH + NP           # o-gate dst partitions incl. out rows (119)

GATES = ("g", "f", "i", "o")
TORCH_BLOCK = {"i": 0, "f": 1, "g": 2, "o": 3}  # torch LSTM gate row blocks

F32 = mybir.dt.float32
F16 = mybir.dt.float16
AF = mybir.ActivationFunctionType


def _lchunk(s_len, n_pairs, n_groups):
    n_chunks = n_pairs * n_groups
    return -(-s_len // n_chunks)  # ceil; tail chunk padded with zero x


# --------------------------------------------------------------------------
# host-side data preparation
# --------------------------------------------------------------------------

def make_weights(wihs, whhs, bihs, bhhs, w_out, b_out):
    """Block-diagonal fp16 stationaries [KR, M] per gate (+ warm variants).

    Rows 0..KH: h rows; KH..KH+2NP: x rows; last row: ones (bias row).
    g-gate has M=KO: cols KH..KH+NP are the fused out-projection, scaled by
    0.5 and decoded host-side with 2*arctanh (b_out baked into the ones-row).
    """
    out = {}
    for t in GATES:
        bi = TORCH_BLOCK[t]
        M = KO if t == "g" else KH
        w = np.zeros((KR, M), np.float32)
        for s in range(NP):
            for d in range(2):
                c0 = 16 * s + 8 * d
                w[c0:c0 + 8, c0:c0 + 8] = whhs[d][8 * bi:8 * bi + 8, :].T
                w[KH + 2 * s + d, c0:c0 + 8] = wihs[d][8 * bi:8 * bi + 8, 0]
                w[KR - 1, c0:c0 + 8] = (bihs[d] + bhhs[d])[8 * bi:8 * bi + 8]
        if t == "g":
            # g rides the merged SIGMOID: tanh(x) = 2*sigmoid(2x)-1, so the
            # whole g block is pre-scaled by 2 and the DVE decodes 2*y-1.
            w *= 2.0
            # fused out-projection: rides the g matmul + the merged sigmoid.
            # sigma(2*out) decoded host-side as 0.5*logit(y).
            for s in range(NP):
                w[16 * s:16 * s + 8, KH + s] = 2.0 * w_out[0, 0:8]
                w[16 * s + 8:16 * s + 16, KH + s] = 2.0 * w_out[0, 8:16]
                w[KR - 1, KH + s] = 2.0 * b_out
        w_warm = w.copy()
        w_warm[:, 0:16] = 0.0  # keep pair-0 (h,c) identically 0 during warmup
        out[f"w_{t}"] = w.astype(np.float16)
        out[f"w_{t}_warm"] = w_warm.astype(np.float16)
    return out


def make_xarr(x_core, future, l_chunk):
    """Per-core x arranged as [G, R, 2*NP+1, BC] fp16; last row is ones."""
    s_len, bc = x_core.shape
    R = l_chunk + W + 1
    xb = x_core[(future - np.arange(s_len)) % s_len]
    xarr = np.zeros((G, R, 2 * NP + 1, bc), np.float32)
    xarr[:, :, 2 * NP, :] = 1.0
    rr = np.arange(R)
    for g in range(G):
        for s in range(NP):
            pos = (g * NP + s) * l_chunk - W + rr
            valid = (pos >= 0) & (pos < s_len)
            for d, src in enumerate((x_core, xb)):
                xarr[g, valid, 2 * s + d, :] = src[pos[valid]]
    return xarr.astype(np.float16)


def make_in_maps(x, wihs, whhs, bihs, bhhs, w_out, b_out, future,
                 use_f32r=None):
    shared = make_weights(wihs, whhs, bihs, bhhs, w_out, float(b_out))
    l_chunk = _lchunk(S, NP, G)
    in_maps = []
    for k in range(N_CORES):
        m = dict(shared)
        m["xarr"] = make_xarr(x[:, k * BC:(k + 1) * BC, 0], future, l_chunk)
        in_maps.append(m)
    return in_maps


# --------------------------------------------------------------------------
# program builder
# --------------------------------------------------------------------------

def build_program(bc=BC, s_len=S, num_devices=N_CORES):
    l_chunk = _lchunk(s_len, NP, G)
    s_pad = l_chunk * NP * G
    R = l_chunk + W + 1

    nc = bacc.Bacc("TRN2", target_bir_lowering=False, debug=False,
                   enable_asserts=False, num_devices=num_devices)

    dram = {}
    host_names = []

    def din(name, shape):
        dram[name] = nc.dram_tensor(name, list(shape), F16, kind="ExternalInput").ap()
        host_names.append(name)

    for t in GATES:
        M = KO if t == "g" else KH
        din(f"w_{t}", (KR, M))
        din(f"w_{t}_warm", (KR, M))
    din("xarr", (G, R, 2 * NP + 1, bc))
    out_d = nc.dram_tensor("out", [s_pad, bc], F16, kind="ExternalOutput").ap()
    out_view = out_d.rearrange("(c l) b -> c l b", l=l_chunk)

    with tile.TileContext(nc) as tc, ExitStack() as ctx:
        consts = ctx.enter_context(tc.tile_pool(name="consts", bufs=1))
        hp = ctx.enter_context(tc.tile_pool(name="hp", bufs=4))
        osb3 = ctx.enter_context(tc.tile_pool(name="osb3", bufs=4))
        cp = ctx.enter_context(tc.tile_pool(name="cp", bufs=3))
        up = ctx.enter_context(tc.tile_pool(name="up", bufs=3))
        zp = ctx.enter_context(tc.tile_pool(name="zp", bufs=3))
        # osb allocates no tiles but its SBUF reservation shifts the layout;
        # removing it reproducibly costs ~220us (1107us -> 1322us), most
        # likely via SBUF bank conflicts between engine streams.  KEEP.
        osb = ctx.enter_context(tc.tile_pool(name="osb", bufs=4))
        osb2 = ctx.enter_context(tc.tile_pool(name="osb2", bufs=4))
        fps = ctx.enter_context(tc.tile_pool(name="fps", bufs=1, space="PSUM"))
        gps = ctx.enter_context(tc.tile_pool(name="gps", bufs=1, space="PSUM"))

        rhs_cur, c_prev = [], []
        ap_ps = []
        # 2 merged PSUM sets [KO, 4, bc] (f,i,o,g+out slices; 4 banks each)
        # shared by G rings via round-parity mapping: ring g, round r ->
        # set (g+r)%2, which balances set load at 1.5 rings/set/round.
        for k in range(2):
            t_ = fps.tile([KO, 4, bc], F32, name=f"aps_{k}", tag=f"aps{k}")
            # partitions KH:KO of the f,i,o slices are never matmul-written;
            # zero once so the merged sigmoid reads clean values there.
            # (compute-engine APs need 32-aligned base partition -> start 96)
            nc.vector.memset(t_[96:KO, 0:3, :], 0.0)
            ap_ps.append(t_)
        for g in range(G):
            r0t = hp.tile([KR, bc], F16, name=f"rhs0_{g}", tag=f"h{g}")
            nc.vector.memset(r0t[0:KH, :], 0.0)
            nc.sync.dma_start(out=r0t[KH:KR, :], in_=dram["xarr"][g, 0])
            c0 = cp.tile([KH, bc], F16, name=f"c0_{g}", tag=f"c{g}")
            nc.vector.memset(c0, 0.0)
            rhs_cur.append(r0t)
            c_prev.append(c0)

        # weight loads after the round-0 rhs tiles so those aren't stuck
        # behind them on the sync queue; warm variants (ring 0's round-0
        # stationaries) ride the otherwise-idle gpsimd queue.
        ct = {}
        for name, ap in dram.items():
            if name == "xarr":
                continue
            t_ = consts.tile(list(ap.shape), ap.dtype, name=f"c_{name}", tag=f"c_{name}")
            (nc.gpsimd if name.endswith("_warm") else nc.sync).dma_start(
                out=t_, in_=ap)
            ct[name] = t_

        u_all = [None] * G

        def front(g, r):
            """matmuls f,i,o,g + ONE merged sigmoid + cell DVE."""
            warm = "_warm" if (g == 0 and r < W) else ""
            rhs = rhs_cur[g]
            ps = (g + r) % 2  # round-parity PSUM set
            if r == R - 1:  # flush round: only the out columns matter
                nc.tensor.matmul(ap_ps[ps][:, 3, :], ct["w_g"], rhs,
                                 start=True, stop=True)
                u_all[g] = up.tile([KO, 4, bc], F16, name=f"ua_{g}_{r}",
                                   tag=f"ua{g}")
                nc.scalar.activation(u_all[g][:, 3, :], ap_ps[ps][:, 3, :],
                                     AF.Sigmoid)
                return
            nc.tensor.matmul(ap_ps[ps][0:KH, 0, :], ct[f"w_f{warm}"], rhs,
                             start=True, stop=True)
            nc.tensor.matmul(ap_ps[ps][0:KH, 1, :], ct[f"w_i{warm}"], rhs,
                             start=True, stop=True)
            nc.tensor.matmul(ap_ps[ps][0:KH, 2, :], ct[f"w_o{warm}"], rhs,
                             start=True, stop=True)
            nc.tensor.matmul(ap_ps[ps][:, 3, :], ct[f"w_g{warm}"], rhs,
                             start=True, stop=True)
            u_all[g] = up.tile([KO, 4, bc], F16, name=f"ua_{g}_{r}", tag=f"ua{g}")
            nc.scalar.activation(u_all[g], ap_ps[ps], AF.Sigmoid)
            # decode tanh(g) = 2*sigmoid(2g) - 1 (g stationaries carry the 2x)
            tg = zp.tile([KH, bc], F16, name=f"tg_{g}_{r}", tag=f"tg{g}")
            nc.vector.tensor_scalar(tg, u_all[g][0:KH, 3, :], 2.0, -1.0,
                                    mybir.AluOpType.mult, mybir.AluOpType.add)
            tm = zp.tile([KH, bc], F16, name=f"t_{g}_{r}", tag=f"tm{g}")
            nc.vector.tensor_mul(tm, u_all[g][0:KH, 0, :], c_prev[g])
            z = zp.tile([KH, bc], F16, name=f"z_{g}_{r}", tag=f"z{g}")
            nc.vector.tensor_mul(z, u_all[g][0:KH, 1, :], tg)
            cn = cp.tile([KH, bc], F16, name=f"c_{g}_{r}", tag=f"c{g}")
            nc.vector.tensor_add(cn, tm, z)
            c_prev[g] = cn

        def back(g, r):
            """tanh(c) + h' + x DMA + out DMA (sigmoid-encoded, from u_all)."""
            if r + 1 < R:
                rhs_n = hp.tile([KR, bc], F16, name=f"rhs_{g}_{r}", tag=f"h{g}")
                nc.sync.dma_start(out=rhs_n[KH:KR, :], in_=dram["xarr"][g, r + 1])
                th = zp.tile([KH, bc], F16, name=f"th_{g}_{r}", tag=f"th{g}")
                nc.scalar.activation(th, c_prev[g], AF.Tanh)
                nc.vector.tensor_mul(rhs_n[0:KH, :], u_all[g][0:KH, 2, :], th)
                rhs_cur[g] = rhs_n
            if r >= W + 1:
                # u_all rows KH:KO of the g slice hold sigmoid(2*(w_out.h+b))
                nc.gpsimd.dma_start(
                    out=out_view[g * NP:(g + 1) * NP, r - 1 - W, :],
                    in_=u_all[g][KH:KO, 3, :])

        for r in range(R):
            if r > 0:
                back(G - 1, r - 1)
            for g in range(G):
                front(g, r)
                if g >= 1:
                    back(g - 1, r)
        back(G - 1, R - 1)

    nc.compile()
    return nc, host_names


# --------------------------------------------------------------------------
# runner
# --------------------------------------------------------------------------

_CACHE = {}


def _get_program(use_f32r=None):
    key = (NP, G, W, BC, S)
    if key not in _CACHE:
        _CACHE[key] = build_program()
    return _CACHE[key]


def kernel(x, w_ih_f, w_hh_f, b_ih_f, b_hh_f, w_ih_b, w_hh_b, b_ih_b, b_hh_b,
           w_out, b_out, future):
    from concourse import bass_utils

    x = np.asarray(x, np.float32)
    wihs = [np.asarray(w_ih_f, np.float32), np.asarray(w_ih_b, np.float32)]
    whhs = [np.asarray(w_hh_f, np.float32), np.asarray(w_hh_b, np.float32)]
    bihs = [np.asarray(b_ih_f, np.float32), np.asarray(b_ih_b, np.float32)]
    bhhs = [np.asarray(b_hh_f, np.float32), np.asarray(b_hh_b, np.float32)]
    w_out = np.asarray(w_out, np.float32)
    b_out = float(np.asarray(b_out).reshape(-1)[0])
    future = int(future)

    nc, names = _get_program()
    in_maps = make_in_maps(x, wihs, whhs, bihs, bhhs, w_out, b_out, future)
    res = bass_utils.run_bass_kernel_spmd(nc, in_maps, core_ids=list(range(N_CORES)))
    out = np.empty((B, S), np.float32)
    for k in range(N_CORES):
        y = np.asarray(res.results[k]["out"][:S, :], np.float32)
        y = np.clip(y, 1e-6, 1.0 - 1e-6)
        # decode sigmoid(2*out): out = 0.5*logit(y)
        out[k * BC:(k + 1) * BC, :] = (0.5 * (np.log(y) - np.log1p(-y))).T
    return out

